# revision 9
# baseline (speedup 1.0000x reference)
"""Trainium2 Bass kernel for nn_BoundaryDiceLoss_82171314307268.

Sharding: pure data-parallel over 8 cores; core c handles sample c//2,
D-half c%2. Each core gets a [H=128(partitions), 70 D-slots, w] slab
(64 owned D slices + 3 halo, out-of-volume D replicated with edge
values).

Host prep (per core):
  v   = (diff > 0) + (63*t + 1) in {1,2,64,65}, bf16, packed
        [128, 70, 132] with replicated w-pad cols 1/130 (cols 0/131
        dead). Base-8-style carry-freedom: the 6-neighbor sum equals
        6*center iff all 6 neighbors equal the center.
  dif = out1 - out0 (owned slots only), bf16 [128, 64*128]
  u   = 1024*t + 1 in {1, 1025}, fp16 [128, 64*128]

Device pipeline (all per-4-slot chunks so Tile software-pipelines):
  E     = (c_v - 6v)^2 via banded 128x128 matmuls (m_b = A1 - 6I with
          replicated H edges) + w/z shifts; ACT Square evacuates PSUM.
  r     = conv3d(E, ball radius 2) decomposed into 8 PE terms:
          T5@E + T3@s3z + T3@f[w-1] + T3@f[w+1]
          + I@E[w-2] + I@E[w+2] + I@E[z-2] + I@E[z+2]
          with s3z = E[z-1]+E[z+1], f = E + s3z (the only 2 DVE adds).
  products with fused row-sum accum (3 STT passes per chunk):
          z1 = (r>0.5)*probs        -> S1 = sum probs*m
          z2 = z1*u                 -> B  = 1024*S2 + S1
          z3 = (r>0.5)*u            -> A  = 1024*S3 + S4
          (A is exact in f32: integer-valued, <= 1024*512+512 < 2^24;
           per-(partition,chunk) decode S3 = A//1024, S4 = A%1024.)
  probs = sigmoid(dif) on ACT.
Host combines [128, 48] f32 partial sums + dice math.
"""
import sys

sys.path.insert(0, "/opt/trn_rl_repo")

import numpy as np
import ml_dtypes

import concourse.bass as bass
import concourse.bacc as bacc
import concourse.tile as tile
import concourse.mybir as mybir
from concourse.bass_utils import run_bass_kernel_spmd

f32 = mybir.dt.float32
bf16 = mybir.dt.bfloat16
fp16 = mybir.dt.float16
Alu = mybir.AluOpType
Act = mybir.ActivationFunctionType

P = 128          # H on partitions
W = 128
OWN = 64         # owned D slices per core
HALO = 3
DEXT = OWN + 2 * HALO          # 70 slab D-slots
WP = W + 4                     # padded w stride, data cols [2, 130)
B = 4
EPS = 1e-05

CH = 4                         # D-slots per chunk (512 free elems)
NE = 17                        # E chunks (slots 1..68)
ND = 16                        # dilation/product chunks (owned 3..66)


def _band(offsets, rep_edges=False):
    m = np.zeros((P, P), np.float32)
    for o in offsets:
        for i in range(P):
            j = i + o
            if 0 <= j < P:
                m[j, i] += 1.0
            elif rep_edges:
                m[min(max(j, 0), P - 1), i] += 1.0
    return m


def _const_mats():
    a1 = _band([-1, 1], rep_edges=True)   # H-neighbor sum, edges replicated
    m_b = a1 - 6.0 * np.eye(P, dtype=np.float32)
    return {"m_b": m_b, "m_t3": _band([-1, 0, 1]),
            "m_t5": _band([-2, -1, 0, 1, 2]),
            "m_id": np.eye(P, dtype=np.float32)}


def _build_program():
    nc = bacc.Bacc("TRN2", target_bir_lowering=False, debug=False,
                   num_devices=8)
    d_v = nc.dram_tensor("v", [P, DEXT * WP], bf16, kind="ExternalInput")
    d_dif = nc.dram_tensor("dif", [P, OWN * W], bf16, kind="ExternalInput")
    d_u = nc.dram_tensor("u", [P, OWN * W], fp16, kind="ExternalInput")
    d_mats = {n: nc.dram_tensor(n, [P, P], bf16, kind="ExternalInput")
              for n in ("m_b", "m_t3", "m_t5", "m_id")}
    d_psums = nc.dram_tensor("psums", [P, 3 * ND], f32, kind="ExternalOutput")

    with tile.TileContext(nc) as tc:
        with tc.tile_pool(name="consts", bufs=1) as cp, \
             tc.tile_pool(name="slabs", bufs=1) as sp, \
             tc.tile_pool(name="scr", bufs=3) as zp, \
             tc.tile_pool(name="ps_e", bufs=3, space="PSUM") as ps_e, \
             tc.tile_pool(name="ps_p", bufs=3, space="PSUM") as ps_p:

            mats = {}
            for n in d_mats:
                mats[n] = cp.tile([P, P], bf16, tag=n, name=n)
                nc.sync.dma_start(mats[n][:], d_mats[n][:])

            def slab(name_, cols=WP, dtype=bf16, slots=DEXT):
                t = sp.tile([P, slots * cols], dtype, tag=name_, name=name_)
                return t, t.rearrange("p (s w) -> p s w", w=cols)

            vflat, v3 = slab("v")
            _, e3 = slab("e")
            _, s3z = slab("s3z")
            _, f3 = slab("f")
            _, tbv = slab("tbv", cols=W, slots=68)
            pflat, probs = slab("probs", cols=W, slots=OWN)
            _, r3 = slab("r", cols=W, slots=OWN)
            uflat, u3 = slab("u", cols=W, dtype=fp16, slots=OWN)
            difflat, dif3 = slab("dif", cols=W, slots=OWN)
            acc = sp.tile([P, 3 * ND], f32, tag="acc")

            # zero E w-pads once (dilation must see 0 out-of-volume;
            # s3z/f inherit zeros from e3's pads automatically)
            nc.vector.memset(e3[:, :, 0:2], 0.0)
            nc.vector.memset(e3[:, :, 130:132], 0.0)

            # ---- input DMA, interleaved so early chunks land first ----

            def dma_v(k):
                s0, s1 = 8 * k, min(8 * (k + 1), DEXT)
                nc.sync.dma_start(vflat[:, s0 * WP:s1 * WP],
                                  d_v[:, s0 * WP:s1 * WP])

            def dma_du(k):
                s0, s1 = 8 * k, 8 * (k + 1)
                nc.sync.dma_start(difflat[:, s0 * W:s1 * W],
                                  d_dif[:, s0 * W:s1 * W])
                nc.sync.dma_start(uflat[:, s0 * W:s1 * W],
                                  d_u[:, s0 * W:s1 * W])

            dma_v(0)
            dma_v(1)
            for k in range(8):
                if k + 2 <= 8:
                    dma_v(k + 2)
                dma_du(k)

            # ---- probs = sigmoid(dif), 8 groups of 8 slots on ACT ----
            for k in range(8):
                s0, s1 = 8 * k * W, 8 * (k + 1) * W
                nc.scalar.activation(pflat[:, s0:s1], difflat[:, s0:s1],
                                     Act.Sigmoid)

            def e_chunk(g):
                sl = slice(1 + CH * g, 5 + CH * g)
                # tbv = v[z-1] + v[z+1]
                g0 = CH * g
                nc.vector.tensor_add(tbv[:, g0:g0 + CH, :],
                                     v3[:, g0:g0 + CH, 2:130],
                                     v3[:, g0 + 2:g0 + CH + 2, 2:130])
                pe_ = ps_e.tile([P, CH * W], f32, tag="eps")
                pe3 = pe_[:].rearrange("p (s w) -> p s w", w=W)
                nc.tensor.matmul(pe3[:], mats["m_b"][:], v3[:, sl, 2:130],
                                 start=True, stop=False)
                nc.tensor.matmul(pe3[:], mats["m_id"][:], v3[:, sl, 1:129],
                                 start=False, stop=False)
                nc.tensor.matmul(pe3[:], mats["m_id"][:], v3[:, sl, 3:131],
                                 start=False, stop=False)
                nc.tensor.matmul(pe3[:], mats["m_id"][:],
                                 tbv[:, g0:g0 + CH, :],
                                 start=False, stop=True)
                nc.scalar.activation(e3[:, sl, 2:130], pe3[:], Act.Square)

            def dil_chunk(j):
                s0 = 3 + CH * j
                sl = slice(s0, s0 + CH)
                nc.vector.tensor_add(s3z[:, sl, :], e3[:, s0 - 1:s0 + CH - 1, :],
                                     e3[:, s0 + 1:s0 + CH + 1, :])
                nc.vector.tensor_add(f3[:, sl, :], e3[:, sl, :],
                                     s3z[:, sl, :])
                pp = ps_p.tile([P, CH * W], f32, tag="pps")
                pp3 = pp[:].rearrange("p (s w) -> p s w", w=W)
                nc.tensor.matmul(pp3[:], mats["m_t5"][:], e3[:, sl, 2:130],
                                 start=True, stop=False)
                nc.tensor.matmul(pp3[:], mats["m_t3"][:], s3z[:, sl, 2:130],
                                 start=False, stop=False)
                nc.tensor.matmul(pp3[:], mats["m_t3"][:], f3[:, sl, 1:129],
                                 start=False, stop=False)
                nc.tensor.matmul(pp3[:], mats["m_t3"][:], f3[:, sl, 3:131],
                                 start=False, stop=False)
                nc.tensor.matmul(pp3[:], mats["m_id"][:], e3[:, sl, 0:128],
                                 start=False, stop=False)
                nc.tensor.matmul(pp3[:], mats["m_id"][:], e3[:, sl, 4:132],
                                 start=False, stop=False)
                nc.tensor.matmul(pp3[:], mats["m_id"][:],
                                 e3[:, s0 - 2:s0 + CH - 2, 2:130],
                                 start=False, stop=False)
                nc.tensor.matmul(pp3[:], mats["m_id"][:],
                                 e3[:, s0 + 2:s0 + CH + 2, 2:130],
                                 start=False, stop=True)
                jj = slice(CH * j, CH * (j + 1))
                nc.scalar.copy(r3[:, jj, :], pp3[:])

            def prod_chunk(j):
                jj = slice(CH * j, CH * (j + 1))
                z1 = zp.tile([P, CH * W], bf16, tag="z1")
                z2 = zp.tile([P, CH * W], fp16, tag="z2")
                z3 = zp.tile([P, CH * W], fp16, tag="z3")
                z13 = z1[:].rearrange("p (s w) -> p s w", w=W)
                nc.vector.scalar_tensor_tensor(
                    z13[:], r3[:, jj, :], 0.5, probs[:, jj, :],
                    op0=Alu.is_gt, op1=Alu.mult,
                    accum_out=acc[:, 3 * j:3 * j + 1])
                nc.vector.scalar_tensor_tensor(
                    z2[:].rearrange("p (s w) -> p s w", w=W),
                    z13[:], 0.0, u3[:, jj, :],
                    op0=Alu.add, op1=Alu.mult,
                    accum_out=acc[:, 3 * j + 1:3 * j + 2])
                nc.vector.scalar_tensor_tensor(
                    z3[:].rearrange("p (s w) -> p s w", w=W),
                    r3[:, jj, :], 0.5, u3[:, jj, :],
                    op0=Alu.is_gt, op1=Alu.mult,
                    accum_out=acc[:, 3 * j + 2:3 * j + 3])

            # skewed emission so engine FIFOs interleave across phases
            for i in range(ND + 3):
                if i < NE:
                    e_chunk(i)
                if 2 <= i < ND + 2:
                    dil_chunk(i - 2)
                if 3 <= i < ND + 3:
                    prod_chunk(i - 3)

            nc.sync.dma_start(d_psums[:], acc[:])

    nc.compile()
    return nc


_CACHE = {}
TRACE = False
TRACE_TMPDIR = None
_LAST = {"exec_time_ns": None, "results": None}


def _get_program():
    if "nc" not in _CACHE:
        _CACHE["nc"] = _build_program()
    return _CACHE["nc"]


def last_exec_time_ns():
    return _LAST["exec_time_ns"]


def _core_slabs(diff_p, tgt_p, c):
    s, h = c // 2, c % 2
    d0 = 0 if h == 0 else OWN
    sl = slice(d0, d0 + DEXT)

    def tr(a):  # [S,H,W] -> [H, S, W]
        return np.ascontiguousarray(a.transpose(1, 0, 2))

    tgt = tr(tgt_p[s][sl])                            # [H, 70, W] f32
    dfull = tr(diff_p[s][sl])                         # [H, 70, W]
    dif = dfull[:, HALO:HALO + OWN]                   # [H, 64, W]
    v = (dfull > 0.0).astype(np.float32) + (63.0 * tgt + 1.0)
    vp = np.zeros((P, DEXT, WP), np.float32)
    vp[:, :, 2:130] = v
    vp[:, :, 1] = v[:, :, 0]
    vp[:, :, 130] = v[:, :, -1]
    u = 1024.0 * tgt[:, HALO:HALO + OWN] + 1.0        # [H, 64, W]
    return {
        "v": vp.reshape(P, DEXT * WP).astype(ml_dtypes.bfloat16),
        "dif": dif.reshape(P, OWN * W).astype(ml_dtypes.bfloat16),
        "u": u.reshape(P, OWN * W).astype(np.float16),
    }


def kernel(output, target):
    output = np.asarray(output, dtype=np.float32)
    target = np.asarray(target, dtype=np.float32)
    nc = _get_program()

    diff = output[:, 1] - output[:, 0]                # [B, D, H, W]
    diff_p = np.pad(diff, ((0, 0), (HALO, HALO), (0, 0), (0, 0)),
                    mode="edge")
    tgt_p = np.pad(target[:, 0], ((0, 0), (HALO, HALO), (0, 0), (0, 0)),
                   mode="edge")

    mats = {n: m.astype(ml_dtypes.bfloat16) for n, m in _const_mats().items()}
    in_maps = []
    for c in range(8):
        m = _core_slabs(diff_p, tgt_p, c)
        m.update(mats)
        in_maps.append(m)

    res = run_bass_kernel_spmd(nc, in_maps, list(range(8)), trace=TRACE,
                               tmpdir=TRACE_TMPDIR)
    _LAST["exec_time_ns"] = res.exec_time_ns
    _LAST["results"] = res

    s1 = np.zeros(B, np.float64)
    s2 = np.zeros(B, np.float64)
    s3 = np.zeros(B, np.float64)
    s4 = np.zeros(B, np.float64)
    for c in range(8):
        a = res.results[c]["psums"].astype(np.float64)   # [128, 48]
        S1 = a[:, 0::3].sum()
        Bv = a[:, 1::3].sum()
        A = a[:, 2::3]
        S3 = np.floor(A / 1024.0).sum()
        S4 = (A - np.floor(A / 1024.0) * 1024.0).sum()
        S2 = (Bv - S1) / 1024.0
        s1[c // 2] += S1
        s2[c // 2] += S2
        s3[c // 2] += S3
        s4[c // 2] += S4
    dice = (2.0 * s2 + EPS) / (s1 + s3 + EPS)
    per_sample = np.where(s4 > 0, 1.0 - dice, 0.0)
    return np.float32(per_sample.sum() / B)
